# revision 69
# baseline (speedup 1.0000x reference)
"""Bass/Tile TRN2 kernel for BantamAttention (sliding-window GQA attention).

Sharding: 8 cores, tensor-parallel on heads. Core c gets q heads 4c..4c+3,
kv head c (Wq/Wk/Wv column slices, per-kv-head cache slice, Wo row slice).
Each core computes a partial (1024, 4096) output (its heads' contribution
through Wo) in bf16; the host sums the 8 partials in f32.

On-device layout: everything is computed "transposed" ([feature, token])
so the PE contraction dim (partitions) lines up with no on-device
transposes of big operands:
  qT/kT = W.T @ hidden.T  (projection matmuls emit [d, token] directly)
  S.T[j,i] = kT.T @ qT    (scores come out keys-on-partitions)
  causal mask for new-key diagonal blocks: PE bias-matmul adds -1e6 into
  the PSUM scores pre-exp (exp -> 0), so no mask tensor or DVE mul.
  softmax denominator: DVE pairwise-add tree over the exp tiles + one
  [1,Q] f32r matmul (instead of 32 ones-matmuls/head on the PE).
  outT[d,i] = V.T-matmul accumulation over j blocks
  partial[i,n] = sum_h oT_h.T @ Wo_h

Projections run as three passes {k,v,q0}, {q1}, {q2,q3} so head-0/1
attention (scores+exp+AV) interleaves with projection matmuls, hiding the
ACT engine's exp time which otherwise gates the attention phase.
New-key blocks skip causally-dead query columns (block t covers columns
128t..1024 only).
"""

import numpy as np
import ml_dtypes
from contextlib import ExitStack

import concourse.bass as bass
from concourse import bacc
import concourse.bass_isa as bass_isa
import concourse.mybir as mybir
import concourse.tile as tile
from concourse.bass_utils import run_bass_kernel_spmd

F32 = mybir.dt.float32
F32R = mybir.dt.float32r
BF16 = mybir.dt.bfloat16
EXP = mybir.ActivationFunctionType.Exp
COPY = mybir.ActivationFunctionType.Copy

Q = 1024          # new tokens
DM = 4096         # model dim
D = 128           # head dim
HPC = 4           # q heads per core
P = 4096          # past length
NCORES = 8
SINKS = 4
NKEEP = 4096      # kept keys (sliding window)
NPAST = 3072      # kept keys that come from the cache (sinks + tail)
NJB = NKEEP // D          # 32 key blocks per head
NJB_PAST = NPAST // D     # 24 from cache, 8 from new tokens
NCHUNK = DM // D          # 32 contraction chunks for projections
SCALE = float(1.0 / np.sqrt(D))
PAST_TAIL0 = P - (NPAST - SINKS)   # 1028: first kept cache row after sinks

TRACE = False
H1_PREPAY = 16
LAST_RESULT = None


def _build():
    nc = bacc.Bacc()
    hT = nc.declare_dram_parameter("hT", [NCHUNK, D, Q], BF16, isOutput=False)
    wqkv0 = nc.declare_dram_parameter("wqkv0", [NCHUNK, D, 3 * D], BF16,
                                      isOutput=False)   # cols [q0, k, v]
    wqkv1 = nc.declare_dram_parameter("wqkv1", [NCHUNK, D, D], BF16,
                                      isOutput=False)   # q1
    wqkv2 = nc.declare_dram_parameter("wqkv2", [NCHUNK, D, 2 * D], BF16,
                                      isOutput=False)   # cols [q2, q3]
    wo = nc.declare_dram_parameter("wo", [D, HPC, DM], BF16, isOutput=False)
    pkT = nc.declare_dram_parameter("pkT", [D, NPAST], BF16, isOutput=False)
    pvp = nc.declare_dram_parameter("pvp", [D, NJB_PAST, D], BF16, isOutput=False)
    cosT = nc.declare_dram_parameter("cosT", [D, Q], BF16, isOutput=False)
    sinE = nc.declare_dram_parameter("sinE", [D, Q], BF16, isOutput=False)
    trib = nc.declare_dram_parameter("trib", [D, D], BF16, isOutput=False)
    identb = nc.declare_dram_parameter("identb", [D, D], BF16, isOutput=False)
    onesc = nc.declare_dram_parameter("onesc", [D, 1], F32R, isOutput=False)
    onesb = nc.declare_dram_parameter("onesb", [D, 1], BF16, isOutput=False)
    onesf = nc.declare_dram_parameter("onesf", [1, D], F32R, isOutput=False)
    outp = nc.declare_dram_parameter("out", [Q, DM], BF16, isOutput=True)

    with ExitStack() as ctx:
        tc = ctx.enter_context(tile.TileContext(nc))
        const = ctx.enter_context(tc.tile_pool(name="const", bufs=1))
        persist = ctx.enter_context(tc.tile_pool(name="persist", bufs=1))

        cos_t = const.tile([D, Q], BF16, tag="cos")
        sin_t = const.tile([D, Q], BF16, tag="sin")
        trib_t = const.tile([D, D], BF16, tag="trib")
        ident_t = const.tile([D, D], BF16, tag="identb")
        onesc_t = const.tile([D, 1], F32R, tag="onesc")
        onesb_t = const.tile([D, 1], BF16, tag="onesb")
        onesf_t = const.tile([1, D], F32R, tag="onesf")

        hres = persist.tile([D, NCHUNK, Q], BF16, tag="hres")
        qT = [persist.tile([D, Q], BF16, tag=f"qT{h}", name=f"qT{h}")
              for h in range(HPC)]
        kT_new = persist.tile([D, Q], BF16, tag="kT_new")
        vT_new = persist.tile([D, Q], BF16, tag="vT_new")
        kT_past = persist.tile([D, NPAST], BF16, tag="kT_past")
        v_keep = persist.tile([D, NJB, D], BF16, tag="v_keep")
        oT = [persist.tile([D, Q], BF16, tag=f"oT{h}", name=f"oT{h}")
              for h in range(HPC)]

        # ---------- shared attention emitters ----------
        # es tiles + den-tree state
        stmp_pool = ctx.enter_context(tc.tile_pool(name="stmp", bufs=4))
        zt = const.tile([D, 512], BF16, tag="warm")

        def warm_into(ap, n):
            # dummy matmuls into a PSUM region that the next real matmul
            # will re-zero (start=True); keeps the PE clock hot across a
            # known stall with no extra PSUM banks
            for _ in range(n):
                nc.tensor.matmul(ap, zt[:, 0:128], zt[:, 0:256],
                                 start=True, stop=True, skip_group_check=True)
        es_pool = esn_pool = l1_pool = acc_pool = rc_pool = bc_pool = None

        state = {}

        def head_begin(h, ps_pool, po_pool, alloc_po=True):
            state[h] = {
                "po": (po_pool.tile([D, Q], F32, tag="po", name=f"po{h}")
                       if alloc_po else None),
                "acc": acc_pool.tile([D, Q], F32R, tag="dacc", name=f"dacc{h}"),
                "es": {},        # jb -> (tile, lo) valid column range [lo, Q)
                "es_new": {},    # t -> (tile, lo), kept alive until norm_den
                "l1": [],        # pending level-1 pair sums
                "nacc": 0,       # number of L1 sums folded into acc
                "ps_pool": ps_pool,
            }

        def emit_scores(h, jb, warm_n=0):
            """scores + exp for block jb of head h; returns nothing (es
            recorded in state). Valid q columns: [lo, Q)."""
            st = state[h]
            ps = st["ps_pool"].tile([D, Q], F32, tag="ps", name=f"ps{h}_{jb}")
            if warm_n:
                warm_into(ps[:, 0:256], warm_n)
            if jb < NJB_PAST:
                lo = 0
                ksl = kT_past[:, jb * D:(jb + 1) * D]
                nc.tensor.matmul(ps[:, 0:512], ksl, qT[h][:, 0:512],
                                 start=True, stop=True)
                nc.tensor.matmul(ps[:, 512:Q], ksl, qT[h][:, 512:Q],
                                 start=True, stop=True)
            else:
                t = jb - NJB_PAST
                lo = t * D
                ksl = kT_new[:, t * D:(t + 1) * D]
                if lo < 512:
                    nc.tensor.matmul(ps[:, lo:512], ksl, qT[h][:, lo:512],
                                     start=True, stop=False,
                                     skip_group_check=True)
                    nc.tensor.matmul(ps[:, 512:Q], ksl, qT[h][:, 512:Q],
                                     start=True, stop=True)
                    nc.tensor.matmul(ps[:, lo:lo + D], trib_t[:, :],
                                     ident_t[:, :], start=False, stop=True,
                                     skip_group_check=True)
                else:
                    nc.tensor.matmul(ps[:, lo:Q], ksl, qT[h][:, lo:Q],
                                     start=True, stop=False,
                                     skip_group_check=True)
                    nc.tensor.matmul(ps[:, lo:lo + D], trib_t[:, :],
                                     ident_t[:, :], start=False, stop=True,
                                     skip_group_check=True)
            if jb < NJB_PAST:
                es = es_pool.tile([D, Q], BF16, tag="es", name=f"es{h}_{jb}")
                esv = es[:, 0:Q]
            else:
                t = jb - NJB_PAST
                es = esn_pool.tile([D, Q - t * D], BF16, tag=f"nt{t}",
                                   name=f"esn{h}_{t}")
                esv = es[:, :]
            nc.scalar.activation(esv, ps[:, lo:Q], EXP, scale=SCALE)
            st["es"][jb] = (esv, lo)

            # denominator: past blocks fold on DVE (distributed over the
            # head's span); new blocks are summed by PE ones-matmuls in
            # emit_norm_den (no serial DVE tail at the head boundary)
            if jb < NJB_PAST:
                if jb % 2 == 1:
                    ea, _ = st["es"][jb - 1]
                    s = l1_pool.tile([D, Q], BF16, tag="l1", name=f"l1_{h}_{jb}")
                    nc.vector.tensor_add(s[:, :], ea[:, :], es[:, :])
                    acc = st["acc"]
                    with nc.allow_low_precision(reason="f32r is fp32-width"):
                        if st["nacc"] == 0:
                            st["l1"].append(s)
                            if len(st["l1"]) == 2:
                                nc.vector.tensor_add(acc[:, :], st["l1"][0][:, :],
                                                     st["l1"][1][:, :])
                                st["l1"] = []
                                st["nacc"] = 2
                        else:
                            nc.vector.tensor_add(acc[:, :], acc[:, :], s[:, :])
                            st["nacc"] += 1
            else:
                st["es_new"][jb - NJB_PAST] = (es, lo)

        def emit_av(h, jb):
            st = state[h]
            if jb < NJB_PAST:
                es, lo = st["es"].pop(jb)
            else:
                es, lo = st["es"][jb]
                st["es_new"][jb - NJB_PAST] = (es, lo)
            po = st["po"]
            vsl = v_keep[:, jb, :]
            stop = jb == NJB - 1
            if lo < 512:
                nc.tensor.matmul(po[:, lo:512], vsl, es[:, 0:512 - lo],
                                 start=(jb == 0), stop=(jb == NJB_PAST + 3),
                                 skip_group_check=True)
                nc.tensor.matmul(po[:, 512:Q], vsl, es[:, 512 - lo:Q - lo],
                                 start=(jb == 0), stop=stop,
                                 skip_group_check=True)
            else:
                nc.tensor.matmul(po[:, lo:Q], vsl, es[:, 0:Q - lo],
                                 start=False, stop=stop,
                                 skip_group_check=True)

        def emit_norm_den(h):
            """den matmul — emit on PE right after last AV of head h."""
            st = state[h]
            pn = st["ps_pool"].tile([D, Q], F32, tag="ps", name=f"pn{h}")
            nc.tensor.matmul(pn[0:1, 0:512], onesc_t[:, :],
                             st["acc"][:, 0:512], start=True, stop=False,
                             skip_group_check=True)
            nc.tensor.matmul(pn[0:1, 512:Q], onesc_t[:, :],
                             st["acc"][:, 512:Q], start=True, stop=False,
                             skip_group_check=True)
            for t in range(8):
                es, lo = st["es_new"].pop(t)
                if lo < 512:
                    nc.tensor.matmul(pn[0:1, lo:512], onesb_t[:, :],
                                     es[:, 0:512 - lo], start=False,
                                     stop=(t == 3), skip_group_check=True)
                    nc.tensor.matmul(pn[0:1, 512:Q], onesb_t[:, :],
                                     es[:, 512 - lo:Q - lo], start=False,
                                     stop=(t == 7), skip_group_check=True)
                else:
                    nc.tensor.matmul(pn[0:1, lo:Q], onesb_t[:, :],
                                     es[:, 0:Q - lo], start=False,
                                     stop=(t == 7), skip_group_check=True)
            rc = rc_pool.tile([1, Q], F32R, tag="rc", name=f"rc{h}")
            with nc.allow_low_precision(reason="f32r is fp32-width"):
                nc.vector.reciprocal(rc[:, :], pn[0:1, :])
            st["pn"] = pn
            st["rc"] = rc

        def emit_norm_bcast(h):
            """bcast + oT mul — emit a bit later so PE doesn't stall on rc."""
            st = state[h]
            pb = st["pn"]
            rc = st["rc"]
            nc.tensor.matmul(pb[:, 0:512], onesf_t[:, :], rc[:, 0:512],
                             start=True, stop=True)
            nc.tensor.matmul(pb[:, 512:Q], onesf_t[:, :], rc[:, 512:Q],
                             start=True, stop=True)
            bc = bc_pool.tile([D, Q], F32, tag="bc", name=f"bc{h}")
            nc.vector.tensor_copy(bc[:, :], pb[:, :])
            nc.vector.tensor_mul(oT[h][:, :], st["po"][:, :], bc[:, :])

        # ---------- P0: projections {k, v, q0} ----------
        with tc.tile_pool(name="p0ps", bufs=6, space="PSUM") as p0ps, \
                tc.tile_pool(name="ptr", bufs=1, space="PSUM") as ptr_pool, \
                tc.tile_pool(name="hin", bufs=4) as hin, \
                tc.tile_pool(name="rtmp", bufs=2) as rtmp:
            # p-state warm-up: dummy matmuls during the DMA fill so the PE
            # clock is at full speed when real projections start
            nc.vector.memset(zt[:, :], 0.0)
            wps_ = ptr_pool.tile([D, D], F32, tag="warmps", name="warmps")
            for _ in range(24):
                nc.tensor.matmul(wps_[:, :], zt[:, 0:128], zt[:, 0:128],
                                 start=True, stop=True)
            accs = [p0ps.tile([D, 512], F32, tag="acc", name=f"p0acc{i}")
                    for i in range(6)]
            Q0, KC, VC = 0, 128, 256   # column offsets in wqkv0
            wgs = {}

            def issue_group(g):
                wg = hin.tile([D, 4, 3 * D], BF16, tag="w0", name=f"w0g{g}")
                if g == 0:
                    for i in range(4):
                        nc.sync.dma_start(wg[:, i, :], wqkv0[i, :, :])
                        nc.sync.dma_start(hres[:, i, :], hT[i, :, :])
                else:
                    nc.sync.dma_start(wg[:, :, :], wqkv0[4 * g:4 * g + 4]
                                      .rearrange("g p c -> p g c"))
                    nc.sync.dma_start(hres[:, 4 * g:4 * g + 4, :],
                                      hT[4 * g:4 * g + 4]
                                      .rearrange("g p c -> p g c"))
                wgs[g] = wg

            issue_group(0)
            issue_group(1)
            issue_group(2)
            issue_group(3)
            for c in range(NCHUNK):
                g = c // 4
                if c % 4 == 0 and g + 4 <= 7:
                    issue_group(g + 4)
                if c == 4:
                    nc.scalar.dma_start(cos_t[:, :], cosT[:, :])
                    nc.scalar.dma_start(sin_t[:, :], sinE[:, :])
                if c == 28:
                    nc.scalar.dma_start(ident_t[:, :], identb[:, :])
                st = c == 0
                sp = c == NCHUNK - 1
                for o, (wlo, hlo) in enumerate(
                        [(Q0, 0), (Q0, 512), (KC, 0), (KC, 512),
                         (VC, 0), (VC, 512)]):
                    nc.tensor.matmul(accs[o][:, :],
                                     wgs[g][:, c % 4, wlo:wlo + D],
                                     hres[:, c, hlo:hlo + 512],
                                     start=st, stop=sp)

            def rope_drain(pacc, dst, s):
                ta = rtmp.tile([D, 512], F32, tag="ropeA", name="ropeA")
                tb = rtmp.tile([D, 512], F32, tag="ropeB", name="ropeB")
                nc.vector.tensor_mul(ta[:, :], pacc[:, :], cos_t[:, s])
                nc.vector.tensor_mul(tb[0:64, :], pacc[64:128, :],
                                     sin_t[64:128, s])
                nc.vector.tensor_mul(tb[64:128, :], pacc[0:64, :],
                                     sin_t[0:64, s])
                nc.vector.tensor_add(dst[:, s], ta[:, :], tb[:, :])

            # ACT drains all accs to SBUF immediately (frees PSUM banks);
            # RoPE runs on DVE from SBUF, off the phase-boundary critical path
            stm = [stmp_pool.tile([D, 512], F32, tag="stmp", name=f"st0_{i}")
                   for i in range(4)]
            nc.scalar.activation(stm[0][:, :], accs[0][:, :], COPY)
            nc.scalar.activation(stm[1][:, :], accs[1][:, :], COPY)
            nc.scalar.activation(vT_new[:, 0:512], accs[4][:, :], COPY)
            nc.scalar.activation(vT_new[:, 512:Q], accs[5][:, :], COPY)
            nc.scalar.activation(stm[2][:, :], accs[2][:, :], COPY)
            nc.scalar.activation(stm[3][:, :], accs[3][:, :], COPY)
            rope_drain(stm[0], qT[0], slice(0, 512))
            rope_drain(stm[1], qT[0], slice(512, Q))
            rope_drain(stm[2], kT_new, slice(0, 512))
            rope_drain(stm[3], kT_new, slice(512, Q))

            # past V into v_keep (single DMA, host-packed [D, 24, D])
            nc.scalar.dma_start(v_keep[:, 0:NJB_PAST, :], pvp[:, :, :])

            # new V transposes (PE) + copies into v_keep
            for t in range(8):
                ptr = ptr_pool.tile([D, D], BF16, tag="ptr", name=f"ptr{t}")
                nc.tensor.transpose(ptr[:, :], vT_new[:, t * D:(t + 1) * D],
                                    ident_t[:, :])
                nc.vector.tensor_copy(v_keep[:, NJB_PAST + t, :], ptr[:, :])

        # ---------- attention pools (span P1a..P3) ----------
        with tc.tile_pool(name="po", bufs=1, space="PSUM") as po_pool, \
                tc.tile_pool(name="es", bufs=22) as es_pool, \
                tc.tile_pool(name="esn", bufs=1) as esn_pool, \
                tc.tile_pool(name="l1", bufs=3) as l1_pool, \
                tc.tile_pool(name="dacc", bufs=2) as acc_pool, \
                tc.tile_pool(name="rc", bufs=2) as rc_pool, \
                tc.tile_pool(name="bc", bufs=1) as bc_pool:
            # ---- P1a: projections {q1} + head-0 past-block attention ----
            with tc.tile_pool(name="p1ps", bufs=2, space="PSUM") as p1ps, \
                    tc.tile_pool(name="ps1", bufs=2, space="PSUM") as ps1, \
                    tc.tile_pool(name="hin1", bufs=2) as hin1, \
                    tc.tile_pool(name="rtmp1", bufs=2) as rtmp1:
                acc1 = [p1ps.tile([D, 512], F32, tag="acc", name=f"p1acc{i}")
                        for i in range(2)]
                nc.scalar.dma_start(kT_past[:, :], pkT[:, :])
                nc.scalar.dma_start(trib_t[:, :], trib[:, :])
                nc.scalar.dma_start(onesc_t[:, :], onesc[:, :])
                nc.scalar.dma_start(onesb_t[:, :], onesb[:, :])
                nc.scalar.dma_start(onesf_t[:, :], onesf[:, :])
                head_begin(0, ps1, po_pool)
                jb_next = 0
                av_next = 0
                wgs1 = {}
                for c in range(NCHUNK):
                    g = c // 4
                    if c % 4 == 0:
                        wg = hin1.tile([D, 4, D], BF16, tag="w1", name=f"w1g{g}")
                        nc.sync.dma_start(wg[:, :, :], wqkv1[4 * g:4 * g + 4]
                                          .rearrange("g p c -> p g c"))
                        wgs1[g] = wg
                    st = c == 0
                    sp = c == NCHUNK - 1
                    nc.tensor.matmul(acc1[0][:, :], wgs1[g][:, c % 4, :],
                                     hres[:, c, 0:512], start=st, stop=sp)
                    nc.tensor.matmul(acc1[1][:, :], wgs1[g][:, c % 4, :],
                                     hres[:, c, 512:Q], start=st, stop=sp)
                    # h0 past-block scores start once q0 rope has drained
                    if c >= 6 and jb_next < NJB_PAST:
                        emit_scores(0, jb_next)
                        jb_next += 1
                        if av_next < jb_next - 1:
                            emit_av(0, av_next)
                            av_next += 1
                def rope_drain1(pacc, dst, s):
                    ta = rtmp1.tile([D, 512], F32, tag="ropeA", name="ropeA")
                    tb = rtmp1.tile([D, 512], F32, tag="ropeB", name="ropeB")
                    nc.vector.tensor_mul(ta[:, :], pacc[:, :], cos_t[:, s])
                    nc.vector.tensor_mul(tb[0:64, :], pacc[64:128, :],
                                         sin_t[64:128, s])
                    nc.vector.tensor_mul(tb[64:128, :], pacc[0:64, :],
                                         sin_t[0:64, s])
                    nc.vector.tensor_add(dst[:, s], ta[:, :], tb[:, :])

                stm1 = [stmp_pool.tile([D, 512], F32, tag="stmp",
                                       name=f"st1_{i}") for i in range(2)]
                nc.scalar.activation(stm1[0][:, :], acc1[0][:, :], COPY)
                nc.scalar.activation(stm1[1][:, :], acc1[1][:, :], COPY)
                rope_drain1(stm1[0], qT[1], slice(0, 512))
                rope_drain1(stm1[1], qT[1], slice(512, Q))
                while jb_next < NJB_PAST:
                    emit_scores(0, jb_next)
                    jb_next += 1
                while av_next < NJB_PAST - 1:
                    emit_av(0, av_next)
                    av_next += 1

            # ---- P1b: projections {q2,q3} + h0 new blocks + h0 finish ----
            with tc.tile_pool(name="p2ps", bufs=4, space="PSUM") as p2ps, \
                    tc.tile_pool(name="ps2", bufs=1, space="PSUM") as ps2, \
                    tc.tile_pool(name="hin2", bufs=2) as hin2, \
                    tc.tile_pool(name="rtmp2", bufs=2) as rtmp2:
                acc2 = [p2ps.tile([D, 512], F32, tag="acc", name=f"p2acc{i}")
                        for i in range(4)]

                state[0]["ps_pool"] = ps2
                jb_next = NJB_PAST
                head_begin(1, ps2, po_pool, alloc_po=False)
                h1_next = 0
                wgs2 = {}
                for c in range(NCHUNK):
                    g = c // 4
                    if c % 4 == 0:
                        wg = hin2.tile([D, 4, 2 * D], BF16, tag="w2",
                                       name=f"w2g{g}")
                        nc.sync.dma_start(wg[:, :, :], wqkv2[4 * g:4 * g + 4]
                                          .rearrange("g p c -> p g c"))
                        wgs2[g] = wg
                    st = c == 0
                    sp = c == NCHUNK - 1
                    for o in range(2):
                        nc.tensor.matmul(acc2[2 * o][:, :],
                                         wgs2[g][:, c % 4, o * D:(o + 1) * D],
                                         hres[:, c, 0:512], start=st, stop=sp)
                        nc.tensor.matmul(acc2[2 * o + 1][:, :],
                                         wgs2[g][:, c % 4, o * D:(o + 1) * D],
                                         hres[:, c, 512:Q], start=st, stop=sp)
                    if c % 3 == 0 and jb_next < NJB:
                        emit_scores(0, jb_next)
                        jb_next += 1
                        emit_av(0, av_next)
                        av_next += 1
                    elif c >= 6 and h1_next < H1_PREPAY:
                        emit_scores(1, h1_next)
                        h1_next += 1
                        if c % 2 == 0 and h1_next < H1_PREPAY:
                            emit_scores(1, h1_next)
                            h1_next += 1
                def rope_drain2(pacc, dst, s, eng):
                    ta = rtmp2.tile([D, 512], F32, tag="ropeA", name="ropeA")
                    tb = rtmp2.tile([D, 512], F32, tag="ropeB", name="ropeB")
                    eng.tensor_mul(ta[:, :], pacc[:, :], cos_t[:, s])
                    eng.tensor_mul(tb[0:64, :], pacc[64:128, :],
                                   sin_t[64:128, s])
                    eng.tensor_mul(tb[64:128, :], pacc[0:64, :],
                                   sin_t[0:64, s])
                    eng.tensor_add(dst[:, s], ta[:, :], tb[:, :])

                stm2 = [stmp_pool.tile([D, 512], F32, tag="stmp",
                                       name=f"st2_{i}") for i in range(4)]
                for i in range(4):
                    nc.scalar.activation(stm2[i][:, :], acc2[i][:, :], COPY)
                rope_drain2(stm2[0], qT[2], slice(0, 512), nc.vector)
                rope_drain2(stm2[1], qT[2], slice(512, Q), nc.vector)
                rope_drain2(stm2[2], qT[3], slice(0, 512), nc.gpsimd)
                rope_drain2(stm2[3], qT[3], slice(512, Q), nc.gpsimd)
                while av_next < NJB:
                    emit_av(0, av_next)
                    av_next += 1
                emit_norm_den(0)
                emit_norm_bcast(0)
                state[1]["h1_next"] = h1_next

            # ---- P3: heads 1..3 ----
            with tc.tile_pool(name="ps3", bufs=3, space="PSUM") as ps3:
                # h1: most past exps prepaid in P1b; drain AVs + finish
                st1 = state[1]
                st1["ps_pool"] = ps3
                st1["po"] = po_pool.tile([D, Q], F32, tag="po", name="po1")
                h1_next = st1.pop("h1_next")
                av1 = 0
                for jb in range(h1_next, NJB):
                    emit_scores(1, jb)
                    for _ in range(3):
                        if av1 < jb - 1:
                            emit_av(1, av1)
                            av1 += 1
                head_begin(2, ps3, po_pool, alloc_po=False)
                pre2 = 0
                while av1 < NJB:
                    emit_av(1, av1)
                    av1 += 1
                    if av1 % 4 == 0 and pre2 < 3:
                        emit_scores(2, pre2)
                        pre2 += 1
                for h in range(2, HPC):
                    if h == 2:
                        st = state[2]
                        st["po"] = po_pool.tile([D, Q], F32, tag="po",
                                                name="po2")
                        first = pre2
                    else:
                        state[3]["po"] = po_pool.tile([D, Q], F32, tag="po",
                                                      name="po3")
                        first = 2
                    for jb in range(first, first + 2):
                        emit_scores(h, jb)
                    emit_norm_den(h - 1)
                    emit_scores(h, first + 2)
                    emit_norm_bcast(h - 1)
                    for jb in range(first + 3, NJB):
                        emit_scores(h, jb)
                        emit_av(h, jb - 3 - first)
                    if h == 2:
                        head_begin(3, ps3, po_pool, alloc_po=False)
                        emit_scores(3, 0)
                    for jb in range(NJB - 3 - first, NJB):
                        emit_av(h, jb)
                        if h == 2 and jb == NJB - 2:
                            emit_scores(3, 1)
                emit_norm_den(HPC - 1)
                emit_norm_bcast(HPC - 1)

        # ---------- P4: output projection ----------
        with tc.tile_pool(name="wo_sb", bufs=2) as wsb, \
                tc.tile_pool(name="out_sb", bufs=2) as osb, \
                tc.tile_pool(name="wo_ps", bufs=6, space="PSUM") as wps:
            wo_tiles = []
            wt = wsb.tile([D, HPC, 512], BF16, tag="wo", name="wo0")
            nc.sync.dma_start(wt[:, :, :], wo[:, :, 0:512])
            wo_tiles.append(wt)
            for nb in range(8):
                if nb + 1 < 8:
                    wt = wsb.tile([D, HPC, 512], BF16, tag="wo",
                                  name=f"wo{nb + 1}")
                    nc.sync.dma_start(wt[:, :, :],
                                      wo[:, :, (nb + 1) * 512:(nb + 2) * 512])
                    wo_tiles.append(wt)
                wo_t = wo_tiles[nb]
                ot = osb.tile([D, 8, 512], BF16, tag="ot", name=f"ot{nb}")
                for ib in range(8):
                    pw = wps.tile([D, 512], F32, tag="pw", name=f"pw{nb}_{ib}")
                    for h in range(HPC):
                        nc.tensor.matmul(pw[:, :], oT[h][:, ib * D:(ib + 1) * D],
                                         wo_t[:, h, :], start=(h == 0),
                                         stop=(h == HPC - 1))
                    if ib % 2 == 0:
                        nc.scalar.activation(ot[:, ib, :], pw[:, :], COPY)
                    else:
                        nc.vector.tensor_copy(ot[:, ib, :], pw[:, :])
                if nb == 7:
                    for ii in range(4):
                        nc.sync.dma_start(
                            outp[ii * 256:(ii + 1) * 256, nb * 512:(nb + 1) * 512]
                            .rearrange("(i p) c -> p i c", p=D),
                            ot[:, 2 * ii:2 * ii + 2, :])
                else:
                    nc.scalar.dma_start(
                        outp[:, nb * 512:(nb + 1) * 512]
                        .rearrange("(i p) c -> p i c", p=D),
                        ot[:, :, :])
    nc.compile()
    return nc


_cache = {}


def kernel(**inputs):
    global LAST_RESULT
    hidden = np.asarray(inputs["hidden"], np.float32)
    Wq = np.asarray(inputs["Wq"], np.float32)
    Wk = np.asarray(inputs["Wk"], np.float32)
    Wv = np.asarray(inputs["Wv"], np.float32)
    Wo = np.asarray(inputs["Wo"], np.float32)
    past_k = np.asarray(inputs["past_k"], np.float32)
    past_v = np.asarray(inputs["past_v"], np.float32)
    cos = np.asarray(inputs["cos"], np.float32)
    sin = np.asarray(inputs["sin"], np.float32)

    bf = ml_dtypes.bfloat16
    hTn = np.ascontiguousarray(hidden[0].T).astype(bf).reshape(NCHUNK, D, Q)
    cosT = np.ascontiguousarray(cos[P:P + Q].T)
    sinT = np.ascontiguousarray(sin[P:P + Q].T)
    sinT[:64] *= -1.0
    sinT = np.ascontiguousarray(np.concatenate([sinT[64:128], sinT[0:64]], axis=0))
    cosT = cosT.astype(bf)
    sinT = sinT.astype(bf)
    tribn = (np.triu(np.ones((D, D), np.float32), 1) * -1e6).astype(bf)
    identn = np.eye(D, dtype=np.float32).astype(bf)
    onescn = np.ones((D, 1), np.float32)
    onesfn = np.ones((1, D), np.float32)

    if "nc" not in _cache:
        _cache["nc"] = _build()
    nc = _cache["nc"]

    in_maps = []
    for c in range(NCORES):
        wq_c = Wq[:, c * HPC * D:(c + 1) * HPC * D]
        wk_c = Wk[:, c * D:(c + 1) * D]
        wv_c = Wv[:, c * D:(c + 1) * D]
        w0_n = np.concatenate([wq_c[:, 0:D], wk_c, wv_c], axis=1).astype(bf)
        w1_n = wq_c[:, D:2 * D].astype(bf)
        w2_n = wq_c[:, 2 * D:4 * D].astype(bf)
        kT = past_k[0, c].T                      # (D, P)
        pkT_n = np.concatenate([kT[:, 0:SINKS], kT[:, PAST_TAIL0:P]],
                               axis=1).astype(bf)            # (D, NPAST)
        pv = past_v[0, c]                        # (P, D)
        pv_pack = np.concatenate([pv[0:SINKS], pv[PAST_TAIL0:P]], axis=0)
        pvp_n = np.ascontiguousarray(
            pv_pack.reshape(NJB_PAST, D, D).transpose(1, 0, 2)).astype(bf)
        in_maps.append({
            "hT": hTn,
            "wqkv0": np.ascontiguousarray(w0_n.reshape(NCHUNK, D, 3 * D)),
            "wqkv1": np.ascontiguousarray(w1_n.reshape(NCHUNK, D, D)),
            "wqkv2": np.ascontiguousarray(w2_n.reshape(NCHUNK, D, 2 * D)),
            "wo": np.ascontiguousarray(
                Wo[c * HPC * D:(c + 1) * HPC * D, :].reshape(HPC, D, DM)
                .transpose(1, 0, 2)).astype(bf),
            "pkT": np.ascontiguousarray(pkT_n),
            "pvp": pvp_n,
            "cosT": cosT,
            "sinE": sinT,
            "trib": tribn,
            "identb": identn,
            "onesc": onescn,
            "onesb": onescn.astype(bf),
            "onesf": onesfn,
        })
    res = run_bass_kernel_spmd(nc, in_maps, list(range(NCORES)), trace=TRACE)
    LAST_RESULT = res
    total = np.zeros((Q, DM), np.float32)
    for r in res.results:
        total += np.asarray(r["out"]).astype(np.float32)
    return total.reshape(1, Q, DM)
